# revision 1
# baseline (speedup 1.0000x reference)
"""Trainium2 Bass kernel for CLS few-shot classifier (Conv4 backbone + cosine head).

Sharding: data-parallel over the 8 episodes (1 task per NeuronCore).
Per core: encode 100 images (25 support + 75 target) through the Conv4
backbone, build class prototypes via the support gram matrix, and emit
cosine-similarity logits [75, 5].

Conv strategy:
  - Images processed in pairs: partitions 0-63 = image A channels,
    64-127 = image B channels; weights are block-diagonal [128, 128].
  - L1 (3->64, 84->42): host-side im2row (K=54 taps*ch + 1 bias/ones row),
    one f32r matmul per PSUM chunk.
  - L2-L4: 9 accumulating matmuls per group (one per 3x3 tap) with
    strided access patterns over zero-padded SBUF activations.
    L3/L4 interleave 3/8 pairs along the row axis so the moving-operand
    free size stays >= 256 (f32r full-rate threshold).
  - PSUM->SBUF evacuation fused with bias+ReLU, split across ScalarE/VectorE.
Head: gram matrix G = E_sup^T E_all via 36 accumulating matmuls over the
[64ch x (100img*36sp)] embedding layout, then prototype dots / norms from G
and a host-built onehot; cosine normalization on [5, 75] logits.
"""

import numpy as np

B, S, T, C = 8, 25, 75, 5
NIMG = S + T          # 100 images per task
NPAIR = NIMG // 2     # 50
TAPS = [(dy, dx) for dy in range(3) for dx in range(3)]
L1_CHUNKS = [(0, 11), (11, 11), (22, 10), (32, 10)]  # (row0, nrows) of 42x42 output

_CACHE = {}


def _build_nc():
    import concourse.bass as bass
    import concourse.mybir as mybir
    import concourse.tile as tile
    from concourse import bacc

    f32 = mybir.dt.float32
    f32r = mybir.dt.float32r
    bf16 = mybir.dt.bfloat16
    AF = mybir.ActivationFunctionType
    ALU = mybir.AluOpType
    AX = mybir.AxisListType

    nc = bacc.Bacc("TRN2", target_bir_lowering=False, debug=False)

    d_im = nc.dram_tensor("im2row", [NPAIR, 55, 1764], bf16, kind="ExternalInput").ap()
    d_wl1 = nc.dram_tensor("wl1", [55, 128], bf16, kind="ExternalInput").ap()
    d_w2 = nc.dram_tensor("w2bd", [128, 9, 128], bf16, kind="ExternalInput").ap()
    d_w3 = nc.dram_tensor("w3bd", [128, 9, 128], bf16, kind="ExternalInput").ap()
    d_w4 = nc.dram_tensor("w4bd", [128, 9, 128], bf16, kind="ExternalInput").ap()
    d_b2 = nc.dram_tensor("b2", [128, 1], f32, kind="ExternalInput").ap()
    d_b3 = nc.dram_tensor("b3", [128, 1], f32, kind="ExternalInput").ap()
    d_b4 = nc.dram_tensor("b4", [128, 1], f32, kind="ExternalInput").ap()
    d_oh5 = nc.dram_tensor("oh5", [25, 5], f32, kind="ExternalInput").ap()
    d_ohT5 = nc.dram_tensor("ohT5", [5, 25], f32, kind="ExternalInput").ap()
    d_out = nc.dram_tensor("preds", [5, 75], f32, kind="ExternalOutput").ap()

    with tile.TileContext(nc) as tc:
        with tc.tile_pool(name="singles", bufs=1) as singles:
            # wl1 is the only DMA the first matmul waits on; it rides the
            # sync (SP) HWDGE FIFO together with the im2row chunks. All other
            # constants ride the scalar (ACT) HWDGE FIFO so they cannot delay
            # the first im2row chunk.
            wl1 = singles.tile([55, 128], bf16, tag="wl1")
            nc.gpsimd.dma_start(out=wl1, in_=d_wl1)
            w2 = singles.tile([128, 9, 128], bf16, tag="w2")
            nc.scalar.dma_start(out=w2, in_=d_w2)
            w3 = singles.tile([128, 9, 128], bf16, tag="w3")
            nc.scalar.dma_start(out=w3, in_=d_w3)
            w4 = singles.tile([128, 9, 128], bf16, tag="w4")
            nc.scalar.dma_start(out=w4, in_=d_w4)
            b2 = singles.tile([128, 1], f32, tag="b2")
            nc.scalar.dma_start(out=b2, in_=d_b2)
            b3 = singles.tile([128, 1], f32, tag="b3")
            nc.scalar.dma_start(out=b3, in_=d_b3)
            b4 = singles.tile([128, 1], f32, tag="b4")
            nc.scalar.dma_start(out=b4, in_=d_b4)
            oh5 = singles.tile([25, 5], f32, tag="oh5")
            nc.scalar.dma_start(out=oh5, in_=d_oh5)
            ohT5 = singles.tile([5, 25], f32, tag="ohT5")
            nc.scalar.dma_start(out=ohT5, in_=d_ohT5)
            ones15 = singles.tile([1, 5], f32, tag="ones15")
            nc.gpsimd.memset(ones15, 1.0)
            ones64 = singles.tile([64, 1], f32, tag="ones64")
            nc.gpsimd.memset(ones64, 1.0)
            warm = singles.tile([1, 1], f32, tag="warm")
            nc.scalar.sqrt(warm, ones15[:, 0:1])

            l2in = [singles.tile([128, 43, 46], bf16, tag=f"l2in{i}", name=f"l2in{i}") for i in range(2)]
            l3in = [singles.tile([128, 23, 68], bf16, tag=f"l3in{i}", name=f"l3in{i}") for i in range(2)]
            l4in = [singles.tile([128, 13, 122], bf16, tag=f"l4in{i}", name=f"l4in{i}") for i in range(2)]
            for t_ in l2in + l3in + l4in:
                nc.gpsimd.memset(t_, 0.0)
            eflat = singles.tile([128, NPAIR, 36], bf16, tag="eflat")
            eall = singles.tile([64, NIMG, 36], bf16, tag="eall")
            sqr = singles.tile([64, NIMG], f32, tag="sqr")

            with tc.tile_pool(name="sqp", bufs=2) as sqp, \
                 tc.tile_pool(name="imp", bufs=3) as imp, \
                 tc.tile_pool(name="pl1", bufs=2, space="PSUM") as pl1, \
                 tc.tile_pool(name="pl2", bufs=2, space="PSUM") as pl2, \
                 tc.tile_pool(name="pl3", bufs=1, space="PSUM") as pl3, \
                 tc.tile_pool(name="pl4", bufs=1, space="PSUM") as pl4:
                next_h = 0
                l3_eng = [0]

                def emit_l1(p, rhs_of):
                    """L1 matmuls for pair p + merged 2-chunk evacuations."""
                    cur2 = l2in[p % 2]
                    col = 0
                    for half in range(2):  # chunk pair (0,1)=rows 0-21, (2,3)=rows 22-41
                        r0, nr = L1_CHUNKS[2 * half][0], L1_CHUNKS[2 * half][1]
                        nr2 = L1_CHUNKS[2 * half + 1][1]
                        nb = nr * 42
                        ps = pl1.tile([128, 2, 512], f32, tag="ps1", name="ps")
                        for j in range(2):
                            nc.tensor.matmul(
                                ps[:, j, :nb],
                                lhsT=wl1,
                                rhs=rhs_of(col, nb),
                                start=True, stop=True,
                            )
                            col += nb
                        src_ = ps[:, :, :nb].rearrange(
                            "p a (r c) -> p a r c", c=42)
                        dst = cur2[:, r0:r0 + nr + nr2, 0:42].rearrange(
                            "p (a r) c -> p a r c", a=2)
                        if half == 0:
                            nc.scalar.activation(dst, src_, AF.Relu)
                        else:
                            nc.vector.tensor_scalar(
                                out=dst, in0=src_, scalar1=0.0, scalar2=None,
                                op0=ALU.max)

                def emit_l4(h):
                    """L4 matmuls for octet h + 2 merged evacuations + de-pair DMAs."""
                    cur4 = l4in[h % 2]
                    nq = min(10, NPAIR - 10 * h)
                    ps4 = pl4.tile([128, 360], f32, tag="ps4", name="ps4")
                    for t, (dy, dx) in enumerate(TAPS):
                        rhs = cur4[:, dy:dy + 11:2, dx:dx + 119:2]
                        nc.tensor.matmul(
                            ps4, lhsT=w4[:, t, :], rhs=rhs,
                            start=(t == 0), stop=(t == 8),
                        )
                    # src view [part, r, q, c]; dst eflat [part, pair, (r c)]
                    src4 = ps4.rearrange("p (r qc) -> p r qc", qc=60)
                    for halfp, eng in ((slice(0, 64), "act"), (slice(64, 128), "dve")):
                        s_ = bass.AP(
                            tensor=src4.tensor, offset=src4.offset,
                            ap=list(src4.ap), const_val=None,
                        )[halfp, :, :].rearrange("p r (q c) -> p q r c", c=6)[:, :nq, :, :]
                        d_ = eflat[halfp, 10 * h:10 * h + nq, :].rearrange(
                            "p q (r c) -> p q r c", c=6)
                        if eng == "act":
                            nc.scalar.activation(d_, s_, AF.Relu, bias=b4[0:64])
                        else:
                            nc.vector.tensor_scalar(
                                out=d_, in0=s_, scalar1=b4[64:128], scalar2=0.0,
                                op0=ALU.add, op1=ALU.max)
                    # de-pair this octet into eall while the conv loop continues
                    nc.sync.dma_start(
                        out=eall[:, 20 * h:20 * h + 2 * nq:2, :],
                        in_=eflat[0:64, 10 * h:10 * h + nq, :])
                    nc.scalar.dma_start(
                        out=eall[:, 20 * h + 1:20 * h + 2 * nq:2, :],
                        in_=eflat[64:128, 10 * h:10 * h + nq, :])
                    esl = eall[:, 20 * h:20 * h + 2 * nq, :]
                    sqt = sqp.tile([64, 20, 36], f32, tag="sqt", name="sqt")
                    nc.vector.tensor_mul(sqt[:, :2 * nq, :], esl, esl)
                    nc.vector.reduce_sum(
                        out=sqr[:, 20 * h:20 * h + 2 * nq],
                        in_=sqt[:, :2 * nq, :], axis=AX.X)

                def emit_l2(p):
                    """L2 for pair p (reads l2in[p%2]) + downstream L3/L4 groups."""
                    nonlocal next_h
                    cur2 = l2in[p % 2]
                    ps2 = pl2.tile([128, 441], f32, tag="ps2", name="ps2")
                    for t, (dy, dx) in enumerate(TAPS):
                        rhs = cur2[:, dy:dy + 41:2, dx:dx + 41:2]
                        nc.tensor.matmul(
                            ps2, lhsT=w2[:, t, :], rhs=rhs,
                            start=(t == 0), stop=(t == 8),
                        )
                    g3, q3 = p // 3, p % 3
                    cur3 = l3in[g3 % 2]
                    src2 = ps2.rearrange("p (r c) -> p r c", c=21)
                    dst3 = cur3[:, 1:22, 22 * q3 + 1:22 * q3 + 22]
                    if p % 2 == 0:
                        nc.scalar.activation(dst3, src2, AF.Relu, bias=b2)
                    else:
                        nc.vector.tensor_scalar(
                            out=dst3, in0=src2, scalar1=b2, scalar2=0.0,
                            op0=ALU.add, op1=ALU.max)
                    # ---- L3 per completed trio ----
                    if q3 == 2 or p == NPAIR - 1:
                        ps3 = pl3.tile([128, 363], f32, tag="ps3", name="ps3")
                        for t, (dy, dx) in enumerate(TAPS):
                            rhs = cur3[:, dy:dy + 21:2, dx:dx + 65:2]
                            nc.tensor.matmul(
                                ps3, lhsT=w3[:, t, :], rhs=rhs,
                                start=(t == 0), stop=(t == 8),
                            )
                        src3 = ps3.rearrange("p (r gc) -> p r gc", gc=33)
                        # group trio pairs by their l4in buffer (octet parity)
                        runs = []
                        for q in range(q3 + 1):
                            pair = 3 * g3 + q
                            h = pair // 10
                            if runs and runs[-1][0] == h:
                                runs[-1][2] += 1
                            else:
                                runs.append([h, q, 1])
                        for h, q0, n in runs:
                            sl0 = (3 * g3 + q0) % 10
                            s_ = src3.rearrange(
                                "p r (q c) -> p q r c", c=11)[:, q0:q0 + n, :, :]
                            d_ = l4in[h % 2][:, 1:12, 12 * sl0:12 * (sl0 + n)]
                            d_ = d_.rearrange(
                                "p r (q c) -> p q r c", c=12)[:, :, :, 1:12]
                            l3_eng[0] ^= 1
                            if l3_eng[0]:
                                nc.scalar.activation(d_, s_, AF.Relu, bias=b3)
                            else:
                                nc.vector.tensor_scalar(
                                    out=d_, in0=s_, scalar1=b3, scalar2=0.0,
                                    op0=ALU.add, op1=ALU.max)
                        # ---- L4 per completed octet ----
                        pe = 3 * g3 + q3
                        while next_h <= (NPAIR - 1) // 10 and (
                                10 * next_h + 9 <= pe or pe == NPAIR - 1):
                            emit_l4(next_h)
                            next_h += 1

                # Software-pipelined emission: L1(p) is issued before L2(p-1)
                # so the tensor engine always has independent work while the
                # previous pair's PSUM is still being evacuated.
                CHUNKS = [1, 2, 3] + [4] * 11   # pair counts per DMA; sum=50
                starts = []
                s0 = 0
                for n in CHUNKS:
                    starts.append(s0)
                    s0 += n
                chunk_of = {}
                for ci, (st, n) in enumerate(zip(starts, CHUNKS)):
                    for q in range(n):
                        chunk_of[st + q] = (ci, st, n)
                imtiles = {}
                for p in range(NPAIR):
                    ci, st, n = chunk_of[p]
                    if p == st:
                        imtile = imp.tile([55, 4, 1764], bf16, tag="im", name="imt")
                        imtiles[ci] = imtile
                        nc.sync.dma_start(
                            out=imtile[:, :n, :],
                            in_=d_im[st:st + n].transpose([1, 0, 2]),
                        )
                    pi = p - st
                    imtile = imtiles[ci]
                    emit_l1(p, lambda col, nb, imtile=imtile, pi=pi:
                            imtile[:, pi, col:col + nb])
                    if p > 0:
                        emit_l2(p - 1)
                emit_l2(NPAIR - 1)

            # ---- head ----
            with tc.tile_pool(name="hs", bufs=1) as hs, \
                 tc.tile_pool(name="ph", bufs=1, space="PSUM") as ph:
                eav = eall.rearrange("p i s -> p (i s)")
                psg = ph.tile([25, 100], f32, tag="g")
                for s in range(36):
                    nc.tensor.matmul(
                        psg,
                        lhsT=eall[:, 0:S, s],
                        rhs=eall[:, :, s],
                        start=(s == 0), stop=(s == 35),
                    )
                gs = hs.tile([25, 100], f32, tag="gs")
                nc.scalar.copy(out=gs, in_=psg)
                psn = ph.tile([1, T], f32, tag="nt")
                nc.tensor.matmul(psn, lhsT=ones64, rhs=sqr[:, S:NIMG],
                                 start=True, stop=True)
                # prototype dots and norms from gram
                psdp = ph.tile([5, T], f32, tag="dp")
                nc.tensor.matmul(psdp, lhsT=oh5, rhs=gs[:, S:NIMG],
                                 start=True, stop=True)
                psa2 = ph.tile([5, S], f32, tag="a2")
                nc.tensor.matmul(psa2, lhsT=oh5, rhs=gs[:, 0:S],
                                 start=True, stop=True)
                a2s = hs.tile([5, S], f32, tag="a2s")
                nc.vector.tensor_mul(a2s, psa2, ohT5)
                np2 = hs.tile([5, 1], f32, tag="np2")
                nc.vector.reduce_sum(out=np2, in_=a2s, axis=AX.X)
                npv = hs.tile([5, 1], f32, tag="npv")
                nc.scalar.sqrt(npv, np2)
                npc_ = hs.tile([5, 1], f32, tag="npc")
                nc.vector.tensor_scalar_max(npc_, npv, 1e-8)
                invp = hs.tile([5, 1], f32, tag="invp")
                nc.vector.reciprocal(invp, npc_)
                ntv = hs.tile([1, T], f32, tag="ntv")
                nc.scalar.sqrt(ntv, psn)
                ntc = hs.tile([1, T], f32, tag="ntc")
                nc.vector.tensor_scalar_max(ntc, ntv, 1e-8)
                invt = hs.tile([1, T], f32, tag="invt")
                nc.vector.reciprocal(invt, ntc)
                psr = ph.tile([5, T], f32, tag="rep")
                nc.tensor.matmul(psr, lhsT=ones15, rhs=invt, start=True, stop=True)
                invtr = hs.tile([5, T], f32, tag="invtr")
                nc.scalar.copy(out=invtr, in_=psr)
                pr1 = hs.tile([5, T], f32, tag="pr1")
                nc.vector.tensor_scalar(
                    out=pr1, in0=psdp, scalar1=invp, scalar2=None, op0=ALU.mult)
                pr2 = hs.tile([5, T], f32, tag="pr2")
                nc.vector.tensor_mul(pr2, pr1, invtr)
                nc.sync.dma_start(out=d_out, in_=pr2)

    nc.compile()
    return nc


def _host_prep(inputs):
    """Build per-core input maps (host-side layout transforms only)."""
    import ml_dtypes
    bf16 = ml_dtypes.bfloat16
    f32 = np.float32
    xs = np.asarray(inputs["x_support_set"], f32)   # [8, 25, 3, 84, 84]
    xt = np.asarray(inputs["x_target_set"], f32)    # [8, 75, 3, 84, 84]
    y = np.asarray(inputs["y_support_set"])         # [8, 25] int32
    W1 = np.asarray(inputs["W1"], f32)
    b1 = np.asarray(inputs["b1"], f32)

    # L1 weights: rows (dy, dx, ci) -> cols co; block diag for the image pair,
    # plus one all-ones row carrying the bias for both halves.
    w1r = W1.transpose(2, 3, 1, 0).reshape(27, 64)
    wl1 = np.zeros((55, 128), f32)
    wl1[0:27, 0:64] = w1r
    wl1[27:54, 64:128] = w1r
    wl1[54, 0:64] = b1
    wl1[54, 64:128] = b1
    wl1 = wl1.astype(bf16)

    def blockdiag(W):
        Wt = W.transpose(2, 3, 1, 0).reshape(9, 64, 64)  # [tap, ci, co]
        bd = np.zeros((9, 128, 128), f32)
        bd[:, 0:64, 0:64] = Wt
        bd[:, 64:128, 64:128] = Wt
        return np.ascontiguousarray(bd.transpose(1, 0, 2))  # [128, 9, 128]

    w2bd = blockdiag(np.asarray(inputs["W2"], f32)).astype(bf16)
    w3bd = blockdiag(np.asarray(inputs["W3"], f32)).astype(bf16)
    w4bd = blockdiag(np.asarray(inputs["W4"], f32)).astype(bf16)
    b2 = np.tile(np.asarray(inputs["b2"], f32), 2).reshape(128, 1)
    b3 = np.tile(np.asarray(inputs["b3"], f32), 2).reshape(128, 1)
    b4 = np.tile(np.asarray(inputs["b4"], f32), 2).reshape(128, 1)

    in_maps = []
    for c in range(B):
        x = np.concatenate([xs[c], xt[c]], 0)  # [100, 3, 84, 84]
        xp = np.zeros((NIMG, 3, 85, 85), f32)
        xp[:, :, :84, :84] = x
        win = np.lib.stride_tricks.sliding_window_view(xp, (3, 3), axis=(2, 3))
        w2v = win[:, :, ::2, ::2, :, :]                  # [100, 3, 42, 42, 3, 3]
        im = w2v.transpose(0, 4, 5, 1, 2, 3).reshape(NIMG, 27, 1764)
        im2row = np.empty((NPAIR, 55, 1764), bf16)
        im2row[:, 0:27] = im[0::2]
        im2row[:, 27:54] = im[1::2]
        im2row[:, 54] = 1.0

        onehot = (np.asarray(y[c]) % C)[:, None] == np.arange(C)[None, :]
        oh5 = (onehot.astype(f32) / C)
        in_maps.append({
            "im2row": im2row,
            "wl1": wl1, "w2bd": w2bd, "w3bd": w3bd, "w4bd": w4bd,
            "b2": b2, "b3": b3, "b4": b4,
            "oh5": np.ascontiguousarray(oh5),
            "ohT5": np.ascontiguousarray(oh5.T),
        })
    return in_maps


def kernel(**inputs):
    from concourse import bass_utils

    if "nc" not in _CACHE:
        _CACHE["nc"] = _build_nc()
    nc = _CACHE["nc"]
    in_maps = _host_prep(inputs)
    res = bass_utils.run_bass_kernel_spmd(nc, in_maps, core_ids=list(range(B)))
    preds = np.stack([r["preds"] for r in res.results], 0)  # [8, 5, 75]
    return np.ascontiguousarray(preds.transpose(0, 2, 1)).astype(np.float32)



# revision 5
# speedup vs baseline: 1.4853x; 1.4853x over previous
"""Trainium2 Bass kernel for CLS few-shot classifier (Conv4 backbone + cosine head).

Sharding: data-parallel over the 8 episodes (1 task per NeuronCore).
Per core: encode 100 images (25 support + 75 target) through the Conv4
backbone, build class prototypes via the support gram matrix, and emit
cosine-similarity logits [75, 5].

Conv strategy (fp8-e4m3 + DoubleRow tensor engine):
  - Images processed in pairs: partitions 0-63 = image A channels,
    64-127 = image B channels; weights are block-diagonal [128, 128].
  - All conv matmuls run in fp8e4 with MatmulPerfMode.DoubleRow: each
    matmul contracts TWO k-tiles (two 3x3 taps, or two halves of the L1
    im2row rows) at 0.5 cycles/output-column - 4x the bf16 MAC rate.
    9 taps pad to 10 (tap 9 = zero weights reading in-bounds padding).
  - L1 (3->64, 84->42): host-side im2row laid out [28, 2, 1764] fp8
    (rows split in two k-tiles); 4 DoubleRow matmuls per image pair.
  - L2-L4: 5 DoubleRow matmuls per conv group over zero-padded fp8 SBUF
    activations; tap-pair rhs APs built manually (ktile stride = tap
    offset delta).
  - PSUM->SBUF evacuation fused with bias+ReLU+fp8-quantize, split
    between ScalarE and VectorE by a greedy load balancer (these two are
    the only engines that can read PSUM; they are the kernel bottleneck).
    L2 evacuates once per TRIO of pairs from a 3-bank PSUM tile to cut
    per-instruction overhead. GPSIMD computes the embedding square-norms
    (SBUF->SBUF) so no evac capacity is spent on them.
Head (bf16): gram matrix G = E_sup^T E_all via 36 accumulating matmuls,
prototype dots / norms from G and a host-built onehot; cosine
normalization on [5, 75] logits.
"""

import numpy as np

B, S, T, C = 8, 25, 75, 5
NIMG = S + T          # 100 images per task
NPAIR = NIMG // 2     # 50
TAPS = [(dy, dx) for dy in range(3) for dx in range(3)]
# tap pairs for DoubleRow: 4 real pairs + (tap8, dummy). The dummy offset
# (2,3) stays in-bounds of every padded activation tile and multiplies
# zero weights.
TAP_PAIRS = [(TAPS[2 * g], TAPS[2 * g + 1]) for g in range(4)] + [(TAPS[8], (2, 3))]
L1_CHUNKS = [(0, 11), (11, 11), (22, 10), (32, 10)]  # (row0, nrows) of 42x42 output

_CACHE = {}


def _build_nc():
    import concourse.bass as bass
    import concourse.mybir as mybir
    import concourse.tile as tile
    from concourse import bacc

    f32 = mybir.dt.float32
    fp8 = mybir.dt.float8e4
    bf16 = mybir.dt.bfloat16
    AF = mybir.ActivationFunctionType
    ALU = mybir.AluOpType
    AX = mybir.AxisListType
    DR = mybir.MatmulPerfMode.DoubleRow

    nc = bacc.Bacc("TRN2", target_bir_lowering=False, debug=False)

    d_im = nc.dram_tensor("im2row", [NPAIR, 28, 3528], fp8, kind="ExternalInput").ap()
    d_wl1 = nc.dram_tensor("wl1", [28, 2, 128], fp8, kind="ExternalInput").ap()
    d_w2 = nc.dram_tensor("w2bd", [128, 10, 128], fp8, kind="ExternalInput").ap()
    d_w3 = nc.dram_tensor("w3bd", [128, 10, 128], fp8, kind="ExternalInput").ap()
    d_w4 = nc.dram_tensor("w4bd", [128, 10, 128], fp8, kind="ExternalInput").ap()
    d_b1 = nc.dram_tensor("b1", [128, 1], f32, kind="ExternalInput").ap()
    d_b2 = nc.dram_tensor("b2", [128, 1], f32, kind="ExternalInput").ap()
    d_b3 = nc.dram_tensor("b3", [128, 1], f32, kind="ExternalInput").ap()
    d_b4 = nc.dram_tensor("b4", [128, 1], f32, kind="ExternalInput").ap()
    d_oh5 = nc.dram_tensor("oh5", [25, 5], f32, kind="ExternalInput").ap()
    d_ohT5 = nc.dram_tensor("ohT5", [5, 25], f32, kind="ExternalInput").ap()
    d_out = nc.dram_tensor("preds", [5, 75], f32, kind="ExternalOutput").ap()

    def dr_rhs(tin, t0, t1, nr, ncol):
        """DoubleRow moving operand: two tap-shifted strided views of a
        padded activation tile stacked on the ktile dim."""
        v = tin[:, :, :]
        rs = v.ap[-2][0]
        cs = v.ap[-1][0]
        (dy0, dx0), (dy1, dx1) = t0, t1
        off0 = dy0 * rs + dx0 * cs
        ks = dy1 * rs + dx1 * cs - off0
        return bass.AP(
            tensor=v.tensor, offset=v.offset + off0,
            ap=[list(v.ap[0]), [ks, 2], [2 * rs, nr], [2 * cs, ncol]],
            const_val=None)

    with tile.TileContext(nc) as tc:
        with tc.tile_pool(name="singles", bufs=1) as singles:
            # wl1 rides the gpsimd/SP HWDGE FIFO ahead of the im2row chunks;
            # the remaining constants trickle one-per-pair on the scalar /
            # vector queues (emitted inside the pair loop just before first
            # use) so they never stall the first evacuations.
            wl1 = singles.tile([28, 2, 128], fp8, tag="wl1")
            nc.gpsimd.dma_start(out=wl1, in_=d_wl1)
            b1 = singles.tile([128, 1], f32, tag="b1")
            nc.scalar.dma_start(out=b1, in_=d_b1)
            w2 = singles.tile([128, 10, 128], fp8, tag="w2")
            w3 = singles.tile([128, 10, 128], fp8, tag="w3")
            w4 = singles.tile([128, 10, 128], fp8, tag="w4")
            b2 = singles.tile([128, 1], f32, tag="b2")
            b3 = singles.tile([128, 1], f32, tag="b3")
            b4 = singles.tile([128, 1], f32, tag="b4")
            oh5 = singles.tile([25, 5], f32, tag="oh5")
            ohT5 = singles.tile([5, 25], f32, tag="ohT5")
            ones15 = singles.tile([1, 5], f32, tag="ones15")
            nc.gpsimd.memset(ones15, 1.0)
            ones64 = singles.tile([64, 1], f32, tag="ones64")
            nc.gpsimd.memset(ones64, 1.0)
            warm = singles.tile([1, 1], f32, tag="warm")
            nc.scalar.sqrt(warm, ones15[:, 0:1])
            late_dmas = {
                1: [(w2, d_w2, "s"), (b2, d_b2, "v")],
                3: [(w3, d_w3, "s"), (b3, d_b3, "v")],
                6: [(w4, d_w4, "s"), (b4, d_b4, "v")],
                12: [(oh5, d_oh5, "s"), (ohT5, d_ohT5, "v")],
            }

            l2in = [singles.tile([128, 43, 46], fp8, tag=f"l2in{i}", name=f"l2in{i}") for i in range(2)]
            l3in = [singles.tile([128, 23, 68], fp8, tag=f"l3in{i}", name=f"l3in{i}") for i in range(2)]
            l4in = [singles.tile([128, 13, 122], fp8, tag=f"l4in{i}", name=f"l4in{i}") for i in range(2)]
            for t_ in l2in + l3in + l4in:
                nc.gpsimd.memset(t_, 0.0)
            eflat = singles.tile([128, NPAIR, 36], bf16, tag="eflat")
            eall = singles.tile([64, NIMG, 36], bf16, tag="eall")
            sqr = singles.tile([64, NIMG], f32, tag="sqr")

            # greedy PSUM-evacuation load balancer over the two engines that
            # can read PSUM
            load = {"act": 0.0, "dve": 0.0}

            def evac(dst, src, bias, elems):
                ca = load["act"] + 0.833 * elems + 143.0
                cd = load["dve"] + 1.0417 * elems + 125.0
                if ca <= cd:
                    load["act"] = ca
                    nc.scalar.activation(dst, src, AF.Relu, bias=bias)
                else:
                    load["dve"] = cd
                    nc.vector.tensor_scalar(
                        out=dst, in0=src, scalar1=bias, scalar2=0.0,
                        op0=ALU.add, op1=ALU.max)

            with tc.tile_pool(name="sqp", bufs=2) as sqp, \
                 tc.tile_pool(name="imp", bufs=3) as imp, \
                 tc.tile_pool(name="pl1", bufs=2, space="PSUM") as pl1, \
                 tc.tile_pool(name="pl2", bufs=1, space="PSUM") as pl2, \
                 tc.tile_pool(name="pl34", bufs=1, space="PSUM") as pl34:
                next_h = 0
                trio_ps = [None]

                def emit_l1(p, imtile, pi):
                    """L1 DoubleRow matmuls for pair p + 2 merged evacuations."""
                    cur2 = l2in[p % 2]
                    col = 0
                    for half in range(2):
                        r0, nr = L1_CHUNKS[2 * half]
                        nr2 = L1_CHUNKS[2 * half + 1][1]
                        nb = nr * 42
                        ps = pl1.tile([128, 2, 512], f32, tag="ps1", name="ps")
                        vi = imtile[:, pi, :]
                        for j in range(2):
                            rhs = bass.AP(
                                tensor=vi.tensor, offset=vi.offset + col,
                                ap=[list(vi.ap[0]), [1764, 2], [1, nb]],
                                const_val=None)
                            nc.tensor.matmul(
                                ps[:, j, :nb],
                                lhsT=wl1,
                                rhs=rhs,
                                start=True, stop=True, perf_mode=DR,
                            )
                            col += nb
                        src_ = ps[:, :, :nb].rearrange(
                            "p a (r c) -> p a r c", c=42)
                        dst = cur2[:, r0:r0 + nr + nr2, 0:42].rearrange(
                            "p (a r) c -> p a r c", a=2)
                        evac(dst, src_, b1, 2 * nb)

                def emit_l4(h):
                    """L4 DoubleRow matmuls for octet h + 1 evacuation +
                    de-pair DMAs + gpsimd square-norms."""
                    cur4 = l4in[h % 2]
                    ps4 = pl34.tile([128, 512], f32, tag="ps34", name="ps4")
                    for g, (t0, t1) in enumerate(TAP_PAIRS):
                        nc.tensor.matmul(
                            ps4[:, 0:360], lhsT=w4[:, 2 * g:2 * g + 2, :],
                            rhs=dr_rhs(cur4, t0, t1, 6, 60),
                            start=(g == 0), stop=(g == 4), perf_mode=DR,
                        )
                    src4 = ps4[:, 0:360].rearrange(
                        "p (r q c) -> p q r c", r=6, c=6)
                    dst4 = eflat[:, 10 * h:10 * h + 10, :].rearrange(
                        "p q (r c) -> p q r c", c=6)
                    evac(dst4, src4, b4, 360)
                    # de-pair this octet into eall while the conv loop continues
                    nc.sync.dma_start(
                        out=eall[:, 20 * h:20 * h + 20:2, :],
                        in_=eflat[0:64, 10 * h:10 * h + 10, :])
                    nc.sync.dma_start(
                        out=eall[:, 20 * h + 1:20 * h + 20:2, :],
                        in_=eflat[64:128, 10 * h:10 * h + 10, :])
                    esl = eall[:, 20 * h:20 * h + 20, :]
                    sqt = sqp.tile([64, 20, 36], f32, tag="sqt", name="sqt")
                    nc.gpsimd.tensor_mul(sqt, esl, esl)
                    load["dve"] += 1.0417 * 720 + 125.0
                    nc.vector.reduce_sum(
                        out=sqr[:, 20 * h:20 * h + 20], in_=sqt, axis=AX.X)

                def emit_l2(p):
                    """L2 DoubleRow matmuls for pair p into the trio PSUM tile;
                    per completed trio: merged evacuation + L3 + L4 cascade."""
                    nonlocal next_h
                    cur2 = l2in[p % 2]
                    g3, q3 = divmod(p, 3)
                    if q3 == 0:
                        trio_ps[0] = pl2.tile([128, 3, 512], f32, tag="ps2",
                                              name="ps2")
                    ps2t = trio_ps[0]
                    for g, (t0, t1) in enumerate(TAP_PAIRS):
                        nc.tensor.matmul(
                            ps2t[:, q3, :441], lhsT=w2[:, 2 * g:2 * g + 2, :],
                            rhs=dr_rhs(cur2, t0, t1, 21, 21),
                            start=(g == 0), stop=(g == 4), perf_mode=DR,
                        )
                    if not (q3 == 2 or p == NPAIR - 1):
                        return
                    nq = q3 + 1
                    cur3 = l3in[g3 % 2]
                    src2 = ps2t[:, :nq, :441].rearrange(
                        "p q (r c) -> p q r c", c=21)
                    dst2 = cur3[:, 1:22, 1:1 + 22 * nq].rearrange(
                        "p r (q c) -> p q r c", c=22)[:, :, :, 0:21]
                    evac(dst2, src2, b2, nq * 441)
                    # ---- L3 for this trio ----
                    ncol3 = 11 * nq
                    ps3 = pl34.tile([128, 512], f32, tag="ps34", name="ps3")
                    for g, (t0, t1) in enumerate(TAP_PAIRS):
                        nc.tensor.matmul(
                            ps3[:, 0:11 * ncol3],
                            lhsT=w3[:, 2 * g:2 * g + 2, :],
                            rhs=dr_rhs(cur3, t0, t1, 11, ncol3),
                            start=(g == 0), stop=(g == 4), perf_mode=DR,
                        )
                    src3 = ps3[:, 0:11 * ncol3].rearrange(
                        "p (r gc) -> p r gc", gc=ncol3)
                    # group trio pairs by their l4in buffer (octet parity)
                    runs = []
                    for q in range(nq):
                        pair = 3 * g3 + q
                        h = pair // 10
                        if runs and runs[-1][0] == h:
                            runs[-1][2] += 1
                        else:
                            runs.append([h, q, 1])
                    for h, q0, n in runs:
                        sl0 = (3 * g3 + q0) % 10
                        s_ = src3.rearrange(
                            "p r (q c) -> p q r c", c=11)[:, q0:q0 + n, :, :]
                        d_ = l4in[h % 2][:, 1:12, 12 * sl0:12 * (sl0 + n)]
                        d_ = d_.rearrange(
                            "p r (q c) -> p q r c", c=12)[:, :, :, 1:12]
                        evac(d_, s_, b3, n * 121)
                    # ---- L4 per completed octet ----
                    pe = 3 * g3 + q3
                    while next_h <= (NPAIR - 1) // 10 and (
                            10 * next_h + 9 <= pe or pe == NPAIR - 1):
                        emit_l4(next_h)
                        next_h += 1

                # Software-pipelined emission: L1(p) is issued before L2(p-1)
                # so the tensor engine always has independent work while the
                # previous pair's PSUM is still being evacuated.
                CHUNKS = [1, 2, 3] + [4] * 11   # pair counts per DMA; sum=50
                starts = []
                s0 = 0
                for n in CHUNKS:
                    starts.append(s0)
                    s0 += n
                chunk_of = {}
                for ci, (st, n) in enumerate(zip(starts, CHUNKS)):
                    for q in range(n):
                        chunk_of[st + q] = (ci, st, n)
                imtiles = {}
                for p in range(NPAIR):
                    ci, st, n = chunk_of[p]
                    if p == st:
                        imtile = imp.tile([28, 4, 3528], fp8, tag="im",
                                          name="imt")
                        imtiles[ci] = imtile
                        nc.sync.dma_start(
                            out=imtile[:, :n, :],
                            in_=d_im[st:st + n].transpose([1, 0, 2]),
                        )
                    imtile = imtiles[ci]
                    emit_l1(p, imtile, p - st)
                    for tdst, tsrc, q in late_dmas.get(p, ()):
                        if q == "s":
                            nc.scalar.dma_start(out=tdst, in_=tsrc)
                        else:
                            nc.gpsimd.dma_start(out=tdst, in_=tsrc)
                    if p > 0:
                        emit_l2(p - 1)
                emit_l2(NPAIR - 1)

            # ---- head ----
            with tc.tile_pool(name="hs", bufs=1) as hs, \
                 tc.tile_pool(name="ph", bufs=1, space="PSUM") as ph:
                psg = ph.tile([25, 100], f32, tag="g")
                for s in range(36):
                    nc.tensor.matmul(
                        psg,
                        lhsT=eall[:, 0:S, s],
                        rhs=eall[:, :, s],
                        start=(s == 0), stop=(s == 35),
                    )
                gs = hs.tile([25, 100], f32, tag="gs")
                nc.scalar.copy(out=gs, in_=psg)
                psn = ph.tile([1, T], f32, tag="nt")
                nc.tensor.matmul(psn, lhsT=ones64, rhs=sqr[:, S:NIMG],
                                 start=True, stop=True)
                # prototype dots and norms from gram
                psdp = ph.tile([5, T], f32, tag="dp")
                nc.tensor.matmul(psdp, lhsT=oh5, rhs=gs[:, S:NIMG],
                                 start=True, stop=True)
                psa2 = ph.tile([5, S], f32, tag="a2")
                nc.tensor.matmul(psa2, lhsT=oh5, rhs=gs[:, 0:S],
                                 start=True, stop=True)
                a2s = hs.tile([5, S], f32, tag="a2s")
                nc.vector.tensor_mul(a2s, psa2, ohT5)
                np2 = hs.tile([5, 1], f32, tag="np2")
                nc.vector.reduce_sum(out=np2, in_=a2s, axis=AX.X)
                npv = hs.tile([5, 1], f32, tag="npv")
                nc.scalar.sqrt(npv, np2)
                npc_ = hs.tile([5, 1], f32, tag="npc")
                nc.vector.tensor_scalar_max(npc_, npv, 1e-8)
                invp = hs.tile([5, 1], f32, tag="invp")
                nc.vector.reciprocal(invp, npc_)
                ntv = hs.tile([1, T], f32, tag="ntv")
                nc.scalar.sqrt(ntv, psn)
                ntc = hs.tile([1, T], f32, tag="ntc")
                nc.vector.tensor_scalar_max(ntc, ntv, 1e-8)
                invt = hs.tile([1, T], f32, tag="invt")
                nc.vector.reciprocal(invt, ntc)
                psr = ph.tile([5, T], f32, tag="rep")
                nc.tensor.matmul(psr, lhsT=ones15, rhs=invt, start=True, stop=True)
                invtr = hs.tile([5, T], f32, tag="invtr")
                nc.scalar.copy(out=invtr, in_=psr)
                pr1 = hs.tile([5, T], f32, tag="pr1")
                nc.vector.tensor_scalar(
                    out=pr1, in0=psdp, scalar1=invp, scalar2=None, op0=ALU.mult)
                pr2 = hs.tile([5, T], f32, tag="pr2")
                nc.vector.tensor_mul(pr2, pr1, invtr)
                nc.sync.dma_start(out=d_out, in_=pr2)

    nc.compile()
    return nc


def _host_prep(inputs):
    """Build per-core input maps (host-side layout transforms only)."""
    import ml_dtypes
    f8 = ml_dtypes.float8_e4m3
    f32 = np.float32
    xs = np.asarray(inputs["x_support_set"], f32)   # [8, 25, 3, 84, 84]
    xt = np.asarray(inputs["x_target_set"], f32)    # [8, 75, 3, 84, 84]
    y = np.asarray(inputs["y_support_set"])         # [8, 25] int32
    W1 = np.asarray(inputs["W1"], f32)

    # L1 weights: rows (dy, dx, ci) -> cols co; block diag for the image
    # pair; rows split into two k-tiles of 28 (rows 54, 55 zero).
    w1r = W1.transpose(2, 3, 1, 0).reshape(27, 64)
    wl1 = np.zeros((56, 128), f32)
    wl1[0:27, 0:64] = w1r
    wl1[27:54, 64:128] = w1r
    wl1 = np.ascontiguousarray(
        wl1.reshape(2, 28, 128).transpose(1, 0, 2)).astype(f8)

    def blockdiag(W):
        Wt = W.transpose(2, 3, 1, 0).reshape(9, 64, 64)  # [tap, ci, co]
        bd = np.zeros((10, 128, 128), f32)
        bd[0:9, 0:64, 0:64] = Wt
        bd[0:9, 64:128, 64:128] = Wt
        return np.ascontiguousarray(bd.transpose(1, 0, 2)).astype(f8)

    w2bd = blockdiag(np.asarray(inputs["W2"], f32))
    w3bd = blockdiag(np.asarray(inputs["W3"], f32))
    w4bd = blockdiag(np.asarray(inputs["W4"], f32))
    b1 = np.tile(np.asarray(inputs["b1"], f32), 2).reshape(128, 1)
    b2 = np.tile(np.asarray(inputs["b2"], f32), 2).reshape(128, 1)
    b3 = np.tile(np.asarray(inputs["b3"], f32), 2).reshape(128, 1)
    b4 = np.tile(np.asarray(inputs["b4"], f32), 2).reshape(128, 1)

    in_maps = []
    for c in range(B):
        x = np.concatenate([xs[c], xt[c]], 0)  # [100, 3, 84, 84]
        xp = np.zeros((NIMG, 3, 85, 85), f32)
        xp[:, :, :84, :84] = x
        win = np.lib.stride_tricks.sliding_window_view(xp, (3, 3), axis=(2, 3))
        w2v = win[:, :, ::2, ::2, :, :]                  # [100, 3, 42, 42, 3, 3]
        im = w2v.transpose(0, 4, 5, 1, 2, 3).reshape(NIMG, 27, 1764)
        im56 = np.zeros((NPAIR, 56, 1764), f32)
        im56[:, 0:27] = im[0::2]
        im56[:, 27:54] = im[1::2]
        im2row = np.ascontiguousarray(
            im56.reshape(NPAIR, 2, 28, 1764).transpose(0, 2, 1, 3)
        ).astype(f8).reshape(NPAIR, 28, 3528)

        onehot = (np.asarray(y[c]) % C)[:, None] == np.arange(C)[None, :]
        oh5 = (onehot.astype(f32) / C)
        in_maps.append({
            "im2row": im2row,
            "wl1": wl1, "w2bd": w2bd, "w3bd": w3bd, "w4bd": w4bd,
            "b1": b1, "b2": b2, "b3": b3, "b4": b4,
            "oh5": np.ascontiguousarray(oh5),
            "ohT5": np.ascontiguousarray(oh5.T),
        })
    return in_maps


def kernel(**inputs):
    from concourse import bass_utils

    if "nc" not in _CACHE:
        _CACHE["nc"] = _build_nc()
    nc = _CACHE["nc"]
    in_maps = _host_prep(inputs)
    res = bass_utils.run_bass_kernel_spmd(nc, in_maps, core_ids=list(range(B)))
    preds = np.stack([r["preds"] for r in res.results], 0)  # [8, 5, 75]
    return np.ascontiguousarray(preds.transpose(0, 2, 1)).astype(np.float32)


# revision 7
# speedup vs baseline: 1.6027x; 1.0790x over previous
"""Trainium2 Bass kernel for CLS few-shot classifier (Conv4 backbone + cosine head).

Sharding: data-parallel over the 8 episodes (1 task per NeuronCore).
Per core: encode 100 images (25 support + 75 target) through the Conv4
backbone, build class prototypes via the support gram matrix, and emit
cosine-similarity logits [75, 5].

Conv strategy (fp8-e4m3 + DoubleRow tensor engine):
  - Images processed in pairs: partitions 0-63 = image A channels,
    64-127 = image B channels; weights are block-diagonal [128, 128].
  - All conv matmuls run in fp8e4 with MatmulPerfMode.DoubleRow: each
    matmul contracts TWO k-tiles (two 3x3 taps, or two halves of the L1
    im2row rows) at 0.5 cycles/output-column - 4x the bf16 MAC rate.
    9 taps pad to 10 (tap 9 = zero weights reading in-bounds padding).
  - L1 (3->64, 84->42): host-side im2row laid out [28, 2, 1764] fp8
    (rows split in two k-tiles); 4 DoubleRow matmuls per image pair.
  - L2-L4: 5 DoubleRow matmuls per conv group over zero-padded fp8 SBUF
    activations; tap-pair rhs APs built manually (ktile stride = tap
    offset delta).
  - PSUM->SBUF evacuation fused with bias+ReLU+fp8-quantize, split
    between ScalarE and VectorE by a greedy load balancer (these two are
    the only engines that can read PSUM; they are the kernel bottleneck).
    L2 evacuates once per TRIO of pairs from a 3-bank PSUM tile to cut
    per-instruction overhead. GPSIMD computes the embedding square-norms
    (SBUF->SBUF) so no evac capacity is spent on them.
Head (bf16): gram matrix G = E_sup^T E_all via 36 accumulating matmuls,
prototype dots / norms from G and a host-built onehot; cosine
normalization on [5, 75] logits.
"""

import numpy as np

B, S, T, C = 8, 25, 75, 5
NIMG = S + T          # 100 images per task
NPAIR = NIMG // 2     # 50
TAPS = [(dy, dx) for dy in range(3) for dx in range(3)]
# tap pairs for DoubleRow: 4 real pairs + (tap8, dummy). The dummy offset
# (2,3) stays in-bounds of every padded activation tile and multiplies
# zero weights.
TAP_PAIRS = [(TAPS[2 * g], TAPS[2 * g + 1]) for g in range(4)] + [(TAPS[8], (2, 3))]
L1_CHUNKS = [(0, 11), (11, 11), (22, 10), (32, 10)]  # (row0, nrows) of 42x42 output

_CACHE = {}


def _build_nc():
    import concourse.bass as bass
    import concourse.mybir as mybir
    import concourse.tile as tile
    from concourse import bacc

    f32 = mybir.dt.float32
    fp8 = mybir.dt.float8e4
    bf16 = mybir.dt.bfloat16
    AF = mybir.ActivationFunctionType
    ALU = mybir.AluOpType
    AX = mybir.AxisListType
    DR = mybir.MatmulPerfMode.DoubleRow

    nc = bacc.Bacc("TRN2", target_bir_lowering=False, debug=False)

    d_im = nc.dram_tensor("im2row", [NPAIR, 28, 3528], fp8, kind="ExternalInput").ap()
    d_wl1 = nc.dram_tensor("wl1", [28, 2, 128], fp8, kind="ExternalInput").ap()
    d_w2 = nc.dram_tensor("w2bd", [128, 10, 128], fp8, kind="ExternalInput").ap()
    d_w3 = nc.dram_tensor("w3bd", [128, 10, 128], fp8, kind="ExternalInput").ap()
    d_w4 = nc.dram_tensor("w4bd", [128, 10, 128], fp8, kind="ExternalInput").ap()
    d_b1 = nc.dram_tensor("b1", [128, 1], f32, kind="ExternalInput").ap()
    d_b2 = nc.dram_tensor("b2", [128, 1], f32, kind="ExternalInput").ap()
    d_b3 = nc.dram_tensor("b3", [128, 1], f32, kind="ExternalInput").ap()
    d_b4 = nc.dram_tensor("b4", [128, 1], f32, kind="ExternalInput").ap()
    d_oh5 = nc.dram_tensor("oh5", [25, 5], f32, kind="ExternalInput").ap()
    d_ohT5 = nc.dram_tensor("ohT5", [5, 25], f32, kind="ExternalInput").ap()
    d_out = nc.dram_tensor("preds", [5, 75], f32, kind="ExternalOutput").ap()

    def dr_rhs(tin, t0, t1, nr, ncol):
        """DoubleRow moving operand: two tap-shifted strided views of a
        padded activation tile stacked on the ktile dim."""
        v = tin[:, :, :]
        rs = v.ap[-2][0]
        cs = v.ap[-1][0]
        (dy0, dx0), (dy1, dx1) = t0, t1
        off0 = dy0 * rs + dx0 * cs
        ks = dy1 * rs + dx1 * cs - off0
        return bass.AP(
            tensor=v.tensor, offset=v.offset + off0,
            ap=[list(v.ap[0]), [ks, 2], [2 * rs, nr], [2 * cs, ncol]],
            const_val=None)

    with tile.TileContext(nc) as tc:
        with tc.tile_pool(name="singles", bufs=1) as singles:
            # wl1 rides the gpsimd/SP HWDGE FIFO ahead of the im2row chunks;
            # the remaining constants trickle one-per-pair on the scalar /
            # vector queues (emitted inside the pair loop just before first
            # use) so they never stall the first evacuations.
            wl1 = singles.tile([28, 2, 128], fp8, tag="wl1")
            nc.gpsimd.dma_start(out=wl1, in_=d_wl1)
            b1 = singles.tile([128, 1], f32, tag="b1")
            nc.scalar.dma_start(out=b1, in_=d_b1)
            w2 = singles.tile([128, 10, 128], fp8, tag="w2")
            w3 = singles.tile([128, 10, 128], fp8, tag="w3")
            w4 = singles.tile([128, 10, 128], fp8, tag="w4")
            b2 = singles.tile([128, 1], f32, tag="b2")
            b3 = singles.tile([128, 1], f32, tag="b3")
            b4 = singles.tile([128, 1], f32, tag="b4")
            oh5 = singles.tile([25, 5], f32, tag="oh5")
            ohT5 = singles.tile([5, 25], f32, tag="ohT5")
            ones15 = singles.tile([1, 5], f32, tag="ones15")
            nc.gpsimd.memset(ones15, 1.0)
            ones64 = singles.tile([64, 1], f32, tag="ones64")
            nc.gpsimd.memset(ones64, 1.0)
            warm = singles.tile([1, 1], f32, tag="warm")
            nc.scalar.sqrt(warm, ones15[:, 0:1])
            late_dmas = {
                1: [(w2, d_w2, "s"), (b2, d_b2, "v")],
                3: [(w3, d_w3, "s"), (b3, d_b3, "v")],
                6: [(w4, d_w4, "s"), (b4, d_b4, "v")],
                12: [(oh5, d_oh5, "s"), (ohT5, d_ohT5, "v")],
            }

            l2in = [singles.tile([128, 43, 46], fp8, tag=f"l2in{i}", name=f"l2in{i}") for i in range(2)]
            l3in = [singles.tile([128, 23, 68], fp8, tag=f"l3in{i}", name=f"l3in{i}") for i in range(2)]
            l4in = [singles.tile([128, 13, 122], fp8, tag=f"l4in{i}", name=f"l4in{i}") for i in range(2)]
            for t_ in l2in + l3in + l4in:
                nc.gpsimd.memset(t_, 0.0)
            eflat = singles.tile([128, NPAIR, 36], bf16, tag="eflat")
            eall = singles.tile([64, NIMG, 36], bf16, tag="eall")
            sqr = singles.tile([64, NIMG], f32, tag="sqr")

            # greedy PSUM-evacuation load balancer over the two engines that
            # can read PSUM
            load = {"act": 0.0, "dve": 0.0}

            def evac(dst, src, bias, elems):
                ca = load["act"] + 0.833 * elems + 143.0
                cd = load["dve"] + 1.0417 * elems + 125.0
                if ca <= cd:
                    load["act"] = ca
                    nc.scalar.activation(dst, src, AF.Relu, bias=bias)
                else:
                    load["dve"] = cd
                    nc.vector.tensor_scalar(
                        out=dst, in0=src, scalar1=bias, scalar2=0.0,
                        op0=ALU.add, op1=ALU.max)

            with tc.tile_pool(name="sqp", bufs=2) as sqp, \
                 tc.tile_pool(name="imp", bufs=3) as imp, \
                 tc.tile_pool(name="pl1", bufs=2, space="PSUM") as pl1, \
                 tc.tile_pool(name="pl2", bufs=2, space="PSUM") as pl2, \
                 tc.tile_pool(name="pl34", bufs=2, space="PSUM") as pl34:
                next_h = 0
                trio_ps = [None]

                def emit_l1(p, imtile, pi):
                    """L1 DoubleRow matmuls for pair p + 2 merged evacuations."""
                    cur2 = l2in[p % 2]
                    col = 0
                    for half in range(2):
                        r0, nr = L1_CHUNKS[2 * half]
                        nr2 = L1_CHUNKS[2 * half + 1][1]
                        nb = nr * 42
                        ps = pl1.tile([128, 2, 512], f32, tag="ps1", name="ps")
                        vi = imtile[:, pi, :]
                        for j in range(2):
                            rhs = bass.AP(
                                tensor=vi.tensor, offset=vi.offset + col,
                                ap=[list(vi.ap[0]), [1764, 2], [1, nb]],
                                const_val=None)
                            nc.tensor.matmul(
                                ps[:, j, :nb],
                                lhsT=wl1,
                                rhs=rhs,
                                start=True, stop=True, perf_mode=DR,
                            )
                            col += nb
                        src_ = ps[:, :, :nb].rearrange(
                            "p a (r c) -> p a r c", c=42)
                        dst = cur2[:, r0:r0 + nr + nr2, 0:42].rearrange(
                            "p (a r) c -> p a r c", a=2)
                        evac(dst, src_, b1, 2 * nb)

                def emit_l4(h):
                    """L4 DoubleRow matmuls for octet h + 1 evacuation +
                    de-pair DMAs + gpsimd square-norms."""
                    cur4 = l4in[h % 2]
                    ps4 = pl34.tile([128, 512], f32, tag="ps34", name="ps4")
                    for g, (t0, t1) in enumerate(TAP_PAIRS):
                        nc.tensor.matmul(
                            ps4[:, 0:360], lhsT=w4[:, 2 * g:2 * g + 2, :],
                            rhs=dr_rhs(cur4, t0, t1, 6, 60),
                            start=(g == 0), stop=(g == 4), perf_mode=DR,
                        )
                    src4 = ps4[:, 0:360].rearrange(
                        "p (r q c) -> p q r c", r=6, c=6)
                    dst4 = eflat[:, 10 * h:10 * h + 10, :].rearrange(
                        "p q (r c) -> p q r c", c=6)
                    evac(dst4, src4, b4, 360)
                    # de-pair this octet into eall while the conv loop continues
                    nc.sync.dma_start(
                        out=eall[:, 20 * h:20 * h + 20:2, :],
                        in_=eflat[0:64, 10 * h:10 * h + 10, :])
                    nc.sync.dma_start(
                        out=eall[:, 20 * h + 1:20 * h + 20:2, :],
                        in_=eflat[64:128, 10 * h:10 * h + 10, :])
                    esl = eall[:, 20 * h:20 * h + 20, :]
                    sqt = sqp.tile([64, 20, 36], f32, tag="sqt", name="sqt")
                    nc.gpsimd.tensor_mul(sqt, esl, esl)
                    load["dve"] += 1.0417 * 720 + 125.0
                    nc.vector.reduce_sum(
                        out=sqr[:, 20 * h:20 * h + 20], in_=sqt, axis=AX.X)

                def emit_l2(p):
                    """L2 DoubleRow matmuls for pair p into the trio PSUM tile;
                    per completed trio: merged evacuation + L3 + L4 cascade."""
                    nonlocal next_h
                    cur2 = l2in[p % 2]
                    g3, q3 = divmod(p, 3)
                    ps2 = pl2.tile([128, 512], f32, tag="ps2", name="ps2")
                    for g, (t0, t1) in enumerate(TAP_PAIRS):
                        nc.tensor.matmul(
                            ps2[:, :441], lhsT=w2[:, 2 * g:2 * g + 2, :],
                            rhs=dr_rhs(cur2, t0, t1, 21, 21),
                            start=(g == 0), stop=(g == 4), perf_mode=DR,
                        )
                    cur3 = l3in[g3 % 2]
                    src2 = ps2[:, :441].rearrange("p (r c) -> p r c", c=21)
                    dst2 = cur3[:, 1:22, 22 * q3 + 1:22 * q3 + 22]
                    evac(dst2, src2, b2, 441)
                    if not (q3 == 2 or p == NPAIR - 1):
                        return
                    nq = q3 + 1
                    # ---- L3 for this trio ----
                    ncol3 = 11 * nq
                    ps3 = pl34.tile([128, 512], f32, tag="ps34", name="ps3")
                    for g, (t0, t1) in enumerate(TAP_PAIRS):
                        nc.tensor.matmul(
                            ps3[:, 0:11 * ncol3],
                            lhsT=w3[:, 2 * g:2 * g + 2, :],
                            rhs=dr_rhs(cur3, t0, t1, 11, ncol3),
                            start=(g == 0), stop=(g == 4), perf_mode=DR,
                        )
                    src3 = ps3[:, 0:11 * ncol3].rearrange(
                        "p (r gc) -> p r gc", gc=ncol3)
                    # group trio pairs by their l4in buffer (octet parity)
                    runs = []
                    for q in range(nq):
                        pair = 3 * g3 + q
                        h = pair // 10
                        if runs and runs[-1][0] == h:
                            runs[-1][2] += 1
                        else:
                            runs.append([h, q, 1])
                    for h, q0, n in runs:
                        sl0 = (3 * g3 + q0) % 10
                        s_ = src3.rearrange(
                            "p r (q c) -> p q r c", c=11)[:, q0:q0 + n, :, :]
                        d_ = l4in[h % 2][:, 1:12, 12 * sl0:12 * (sl0 + n)]
                        d_ = d_.rearrange(
                            "p r (q c) -> p q r c", c=12)[:, :, :, 1:12]
                        evac(d_, s_, b3, n * 121)
                    # ---- L4 per completed octet ----
                    pe = 3 * g3 + q3
                    while next_h <= (NPAIR - 1) // 10 and (
                            10 * next_h + 9 <= pe or pe == NPAIR - 1):
                        emit_l4(next_h)
                        next_h += 1

                # Software-pipelined emission: L1(p) is issued before L2(p-1)
                # so the tensor engine always has independent work while the
                # previous pair's PSUM is still being evacuated.
                CHUNKS = [1, 2, 3] + [4] * 11   # pair counts per DMA; sum=50
                starts = []
                s0 = 0
                for n in CHUNKS:
                    starts.append(s0)
                    s0 += n
                chunk_of = {}
                for ci, (st, n) in enumerate(zip(starts, CHUNKS)):
                    for q in range(n):
                        chunk_of[st + q] = (ci, st, n)
                imtiles = {}

                def fetch_chunk(ci):
                    if ci >= len(CHUNKS) or ci in imtiles:
                        return
                    st, n = starts[ci], CHUNKS[ci]
                    imtile = imp.tile([28, 4, 3528], fp8, tag="im",
                                      name="imt")
                    imtiles[ci] = imtile
                    nc.sync.dma_start(
                        out=imtile[:, :n, :],
                        in_=d_im[st:st + n].transpose([1, 0, 2]),
                    )

                for p in range(NPAIR):
                    ci, st, n = chunk_of[p]
                    fetch_chunk(ci)
                    if p == st:
                        fetch_chunk(ci + 1)
                    imtile = imtiles[ci]
                    emit_l1(p, imtile, p - st)
                    for tdst, tsrc, q in late_dmas.get(p, ()):
                        if q == "s":
                            nc.scalar.dma_start(out=tdst, in_=tsrc)
                        else:
                            nc.gpsimd.dma_start(out=tdst, in_=tsrc)
                    if p > 0:
                        emit_l2(p - 1)
                emit_l2(NPAIR - 1)

            # ---- head ----
            with tc.tile_pool(name="hs", bufs=1) as hs, \
                 tc.tile_pool(name="ph", bufs=1, space="PSUM") as ph:
                psg = ph.tile([25, 100], f32, tag="g")
                for s in range(36):
                    nc.tensor.matmul(
                        psg,
                        lhsT=eall[:, 0:S, s],
                        rhs=eall[:, :, s],
                        start=(s == 0), stop=(s == 35),
                    )
                gs = hs.tile([25, 100], f32, tag="gs")
                nc.scalar.copy(out=gs, in_=psg)
                psn = ph.tile([1, T], f32, tag="nt")
                nc.tensor.matmul(psn, lhsT=ones64, rhs=sqr[:, S:NIMG],
                                 start=True, stop=True)
                # prototype dots and norms from gram
                psdp = ph.tile([5, T], f32, tag="dp")
                nc.tensor.matmul(psdp, lhsT=oh5, rhs=gs[:, S:NIMG],
                                 start=True, stop=True)
                psa2 = ph.tile([5, S], f32, tag="a2")
                nc.tensor.matmul(psa2, lhsT=oh5, rhs=gs[:, 0:S],
                                 start=True, stop=True)
                a2s = hs.tile([5, S], f32, tag="a2s")
                nc.vector.tensor_mul(a2s, psa2, ohT5)
                np2 = hs.tile([5, 1], f32, tag="np2")
                nc.vector.reduce_sum(out=np2, in_=a2s, axis=AX.X)
                npv = hs.tile([5, 1], f32, tag="npv")
                nc.scalar.sqrt(npv, np2)
                npc_ = hs.tile([5, 1], f32, tag="npc")
                nc.vector.tensor_scalar_max(npc_, npv, 1e-8)
                invp = hs.tile([5, 1], f32, tag="invp")
                nc.vector.reciprocal(invp, npc_)
                ntv = hs.tile([1, T], f32, tag="ntv")
                nc.scalar.sqrt(ntv, psn)
                ntc = hs.tile([1, T], f32, tag="ntc")
                nc.vector.tensor_scalar_max(ntc, ntv, 1e-8)
                invt = hs.tile([1, T], f32, tag="invt")
                nc.vector.reciprocal(invt, ntc)
                psr = ph.tile([5, T], f32, tag="rep")
                nc.tensor.matmul(psr, lhsT=ones15, rhs=invt, start=True, stop=True)
                invtr = hs.tile([5, T], f32, tag="invtr")
                nc.scalar.copy(out=invtr, in_=psr)
                pr1 = hs.tile([5, T], f32, tag="pr1")
                nc.vector.tensor_scalar(
                    out=pr1, in0=psdp, scalar1=invp, scalar2=None, op0=ALU.mult)
                pr2 = hs.tile([5, T], f32, tag="pr2")
                nc.vector.tensor_mul(pr2, pr1, invtr)
                nc.sync.dma_start(out=d_out, in_=pr2)

    nc.compile()
    return nc


def _host_prep(inputs):
    """Build per-core input maps (host-side layout transforms only)."""
    import ml_dtypes
    f8 = ml_dtypes.float8_e4m3
    f32 = np.float32
    xs = np.asarray(inputs["x_support_set"], f32)   # [8, 25, 3, 84, 84]
    xt = np.asarray(inputs["x_target_set"], f32)    # [8, 75, 3, 84, 84]
    y = np.asarray(inputs["y_support_set"])         # [8, 25] int32
    W1 = np.asarray(inputs["W1"], f32)

    # L1 weights: rows (dy, dx, ci) -> cols co; block diag for the image
    # pair; rows split into two k-tiles of 28 (rows 54, 55 zero).
    w1r = W1.transpose(2, 3, 1, 0).reshape(27, 64)
    wl1 = np.zeros((56, 128), f32)
    wl1[0:27, 0:64] = w1r
    wl1[27:54, 64:128] = w1r
    wl1 = np.ascontiguousarray(
        wl1.reshape(2, 28, 128).transpose(1, 0, 2)).astype(f8)

    def blockdiag(W):
        Wt = W.transpose(2, 3, 1, 0).reshape(9, 64, 64)  # [tap, ci, co]
        bd = np.zeros((10, 128, 128), f32)
        bd[0:9, 0:64, 0:64] = Wt
        bd[0:9, 64:128, 64:128] = Wt
        return np.ascontiguousarray(bd.transpose(1, 0, 2)).astype(f8)

    w2bd = blockdiag(np.asarray(inputs["W2"], f32))
    w3bd = blockdiag(np.asarray(inputs["W3"], f32))
    w4bd = blockdiag(np.asarray(inputs["W4"], f32))
    b1 = np.tile(np.asarray(inputs["b1"], f32), 2).reshape(128, 1)
    b2 = np.tile(np.asarray(inputs["b2"], f32), 2).reshape(128, 1)
    b3 = np.tile(np.asarray(inputs["b3"], f32), 2).reshape(128, 1)
    b4 = np.tile(np.asarray(inputs["b4"], f32), 2).reshape(128, 1)

    in_maps = []
    for c in range(B):
        x = np.concatenate([xs[c], xt[c]], 0)  # [100, 3, 84, 84]
        xp = np.zeros((NIMG, 3, 85, 85), f32)
        xp[:, :, :84, :84] = x
        win = np.lib.stride_tricks.sliding_window_view(xp, (3, 3), axis=(2, 3))
        w2v = win[:, :, ::2, ::2, :, :]                  # [100, 3, 42, 42, 3, 3]
        im = w2v.transpose(0, 4, 5, 1, 2, 3).reshape(NIMG, 27, 1764)
        im56 = np.zeros((NPAIR, 56, 1764), f32)
        im56[:, 0:27] = im[0::2]
        im56[:, 27:54] = im[1::2]
        im2row = np.ascontiguousarray(
            im56.reshape(NPAIR, 2, 28, 1764).transpose(0, 2, 1, 3)
        ).astype(f8).reshape(NPAIR, 28, 3528)

        onehot = (np.asarray(y[c]) % C)[:, None] == np.arange(C)[None, :]
        oh5 = (onehot.astype(f32) / C)
        in_maps.append({
            "im2row": im2row,
            "wl1": wl1, "w2bd": w2bd, "w3bd": w3bd, "w4bd": w4bd,
            "b1": b1, "b2": b2, "b3": b3, "b4": b4,
            "oh5": np.ascontiguousarray(oh5),
            "ohT5": np.ascontiguousarray(oh5.T),
        })
    return in_maps


def kernel(**inputs):
    from concourse import bass_utils

    if "nc" not in _CACHE:
        _CACHE["nc"] = _build_nc()
    nc = _CACHE["nc"]
    in_maps = _host_prep(inputs)
    res = bass_utils.run_bass_kernel_spmd(nc, in_maps, core_ids=list(range(B)))
    preds = np.stack([r["preds"] for r in res.results], 0)  # [8, 5, 75]
    return np.ascontiguousarray(preds.transpose(0, 2, 1)).astype(np.float32)


# revision 10
# speedup vs baseline: 1.6143x; 1.0072x over previous
"""Trainium2 Bass kernel for CLS few-shot classifier (Conv4 backbone + cosine head).

Sharding: data-parallel over the 8 episodes (1 task per NeuronCore).
Per core: encode 100 images (25 support + 75 target) through the Conv4
backbone, build class prototypes via the support gram matrix, and emit
cosine-similarity logits [75, 5].

Conv strategy (fp8-e4m3 + DoubleRow tensor engine):
  - Images processed in pairs: partitions 0-63 = image A channels,
    64-127 = image B channels; weights are block-diagonal [128, 128].
  - All conv matmuls run in fp8e4 with MatmulPerfMode.DoubleRow: each
    matmul contracts TWO k-tiles (two 3x3 taps, or two halves of the L1
    im2row rows) at 0.5 cycles/output-column - 4x the bf16 MAC rate.
    9 taps pad to 10 (tap 9 = zero weights reading in-bounds padding).
  - L1 (3->64, 84->42): host-side im2row laid out [28, 2, 1764] fp8
    (rows split in two k-tiles); 4 DoubleRow matmuls per image pair.
  - L2-L4: 5 DoubleRow matmuls per conv group over zero-padded fp8 SBUF
    activations; tap-pair rhs APs built manually (ktile stride = tap
    offset delta).
  - PSUM->SBUF evacuation fused with bias+ReLU+fp8-quantize, split
    between ScalarE and VectorE by a greedy load balancer (these two are
    the only engines that can read PSUM; they are the kernel bottleneck).
    L2 evacuates once per TRIO of pairs from a 3-bank PSUM tile to cut
    per-instruction overhead. GPSIMD computes the embedding square-norms
    (SBUF->SBUF) so no evac capacity is spent on them.
Head (bf16): gram matrix G = E_sup^T E_all via 36 accumulating matmuls,
prototype dots / norms from G and a host-built onehot; cosine
normalization on [5, 75] logits.
"""

import numpy as np

B, S, T, C = 8, 25, 75, 5
NIMG = S + T          # 100 images per task
NPAIR = NIMG // 2     # 50
TAPS = [(dy, dx) for dy in range(3) for dx in range(3)]
# tap pairs for DoubleRow: 4 real pairs + (tap8, dummy). The dummy offset
# (2,3) stays in-bounds of every padded activation tile and multiplies
# zero weights.
TAP_PAIRS = [(TAPS[2 * g], TAPS[2 * g + 1]) for g in range(4)] + [(TAPS[8], (2, 3))]
L1_CHUNKS = [(0, 11), (11, 11), (22, 10), (32, 10)]  # (row0, nrows) of 42x42 output

_CACHE = {}


def _build_nc():
    import concourse.bass as bass
    import concourse.mybir as mybir
    import concourse.tile as tile
    from concourse import bacc

    f32 = mybir.dt.float32
    fp8 = mybir.dt.float8e4
    bf16 = mybir.dt.bfloat16
    AF = mybir.ActivationFunctionType
    ALU = mybir.AluOpType
    AX = mybir.AxisListType
    DR = mybir.MatmulPerfMode.DoubleRow

    nc = bacc.Bacc("TRN2", target_bir_lowering=False, debug=False)

    d_im = nc.dram_tensor("im2row", [NPAIR, 28, 3528], fp8, kind="ExternalInput").ap()
    d_wl1 = nc.dram_tensor("wl1", [28, 2, 128], fp8, kind="ExternalInput").ap()
    d_w2 = nc.dram_tensor("w2bd", [128, 10, 128], fp8, kind="ExternalInput").ap()
    d_w3 = nc.dram_tensor("w3bd", [128, 10, 128], fp8, kind="ExternalInput").ap()
    d_w4 = nc.dram_tensor("w4bd", [128, 10, 128], fp8, kind="ExternalInput").ap()
    d_b1 = nc.dram_tensor("b1", [128, 1], f32, kind="ExternalInput").ap()
    d_b2 = nc.dram_tensor("b2", [128, 1], f32, kind="ExternalInput").ap()
    d_b3 = nc.dram_tensor("b3", [128, 1], f32, kind="ExternalInput").ap()
    d_b4 = nc.dram_tensor("b4", [128, 1], f32, kind="ExternalInput").ap()
    d_oh5 = nc.dram_tensor("oh5", [25, 5], f32, kind="ExternalInput").ap()
    d_ohT5 = nc.dram_tensor("ohT5", [5, 25], f32, kind="ExternalInput").ap()
    d_out = nc.dram_tensor("preds", [5, 75], f32, kind="ExternalOutput").ap()

    def dr_rhs(tin, t0, t1, nr, ncol, col0=0):
        """DoubleRow moving operand: two tap-shifted strided views of a
        padded activation tile stacked on the ktile dim."""
        v = tin[:, :, :]
        rs = v.ap[-2][0]
        cs = v.ap[-1][0]
        (dy0, dx0), (dy1, dx1) = t0, t1
        off0 = dy0 * rs + (dx0 + col0) * cs
        ks = (dy1 - dy0) * rs + (dx1 - dx0) * cs
        return bass.AP(
            tensor=v.tensor, offset=v.offset + off0,
            ap=[list(v.ap[0]), [ks, 2], [2 * rs, nr], [2 * cs, ncol]],
            const_val=None)

    with tile.TileContext(nc) as tc:
        with tc.tile_pool(name="singles", bufs=1) as singles:
            # wl1 rides the gpsimd/SP HWDGE FIFO ahead of the im2row chunks;
            # the remaining constants trickle one-per-pair on the scalar /
            # vector queues (emitted inside the pair loop just before first
            # use) so they never stall the first evacuations.
            wl1 = singles.tile([28, 2, 128], fp8, tag="wl1")
            nc.gpsimd.dma_start(out=wl1, in_=d_wl1)
            b1 = singles.tile([128, 1], f32, tag="b1")
            nc.scalar.dma_start(out=b1, in_=d_b1)
            w2 = singles.tile([128, 10, 128], fp8, tag="w2")
            w3 = singles.tile([128, 10, 128], fp8, tag="w3")
            w4 = singles.tile([128, 10, 128], fp8, tag="w4")
            b2 = singles.tile([128, 1], f32, tag="b2")
            b3 = singles.tile([128, 1], f32, tag="b3")
            b4 = singles.tile([128, 1], f32, tag="b4")
            oh5 = singles.tile([25, 5], f32, tag="oh5")
            ohT5 = singles.tile([5, 25], f32, tag="ohT5")
            ones15 = singles.tile([1, 5], f32, tag="ones15")
            nc.gpsimd.memset(ones15, 1.0)
            ones64 = singles.tile([64, 1], f32, tag="ones64")
            nc.gpsimd.memset(ones64, 1.0)
            warm = singles.tile([1, 1], f32, tag="warm")
            nc.scalar.sqrt(warm, ones15[:, 0:1])
            late_dmas = {
                1: [(w2, d_w2, "s"), (b2, d_b2, "v")],
                3: [(w3, d_w3, "s"), (b3, d_b3, "v")],
                6: [(w4, d_w4, "s"), (b4, d_b4, "v")],
                12: [(oh5, d_oh5, "s"), (ohT5, d_ohT5, "v")],
            }

            l2in = [singles.tile([128, 43, 46], fp8, tag=f"l2in{i}", name=f"l2in{i}") for i in range(2)]
            l3in = [singles.tile([128, 23, 68], fp8, tag=f"l3in{i}", name=f"l3in{i}") for i in range(2)]
            l4in = [singles.tile([128, 13, 122], fp8, tag=f"l4in{i}", name=f"l4in{i}") for i in range(2)]
            for t_ in l2in + l3in + l4in:
                nc.gpsimd.memset(t_, 0.0)
            eflat = singles.tile([128, NPAIR, 36], bf16, tag="eflat")
            gs = singles.tile([25, 100], f32, tag="gs")
            np2 = singles.tile([5, 1], f32, tag="np2")
            invp = singles.tile([5, 1], f32, tag="invp")
            eall = singles.tile([64, NIMG, 36], bf16, tag="eall")
            sqr = singles.tile([64, NIMG], f32, tag="sqr")

            # greedy PSUM-evacuation load balancer over the two engines that
            # can read PSUM
            load = {"act": 0.0, "dve": 0.0}

            def evac(dst, src, bias, elems):
                ca = load["act"] + 0.833 * elems + 143.0
                cd = load["dve"] + 1.0417 * elems + 125.0
                if ca <= cd:
                    load["act"] = ca
                    nc.scalar.activation(dst, src, AF.Relu, bias=bias)
                else:
                    load["dve"] = cd
                    nc.vector.tensor_scalar(
                        out=dst, in0=src, scalar1=bias, scalar2=0.0,
                        op0=ALU.add, op1=ALU.max)

            with tc.tile_pool(name="sqp", bufs=2) as sqp, \
                 tc.tile_pool(name="imp", bufs=3) as imp, \
                 tc.tile_pool(name="pl1", bufs=2, space="PSUM") as pl1, \
                 tc.tile_pool(name="pl2", bufs=2, space="PSUM") as pl2, \
                 tc.tile_pool(name="pl34", bufs=1, space="PSUM") as pl34, \
                 tc.tile_pool(name="pg", bufs=1, space="PSUM") as pg:
                next_h = 0
                gram_ps = [None]
                L4_RANGES = [(0, 10, 9), (10, 10, 19), (20, 10, 29),
                             (30, 10, 39), (40, 5, 44), (45, 5, 49)]

                def emit_l1(p, imtile, pi):
                    """L1 DoubleRow matmuls for pair p + 2 merged evacuations."""
                    cur2 = l2in[p % 2]
                    col = 0
                    for half in range(2):
                        r0, nr = L1_CHUNKS[2 * half]
                        nr2 = L1_CHUNKS[2 * half + 1][1]
                        nb = nr * 42
                        ps = pl1.tile([128, 2, 512], f32, tag="ps1", name="ps")
                        vi = imtile[:, pi, :]
                        for j in range(2):
                            rhs = bass.AP(
                                tensor=vi.tensor, offset=vi.offset + col,
                                ap=[list(vi.ap[0]), [1764, 2], [1, nb]],
                                const_val=None)
                            nc.tensor.matmul(
                                ps[:, j, :nb],
                                lhsT=wl1,
                                rhs=rhs,
                                start=True, stop=True, perf_mode=DR,
                            )
                            col += nb
                        src_ = ps[:, :, :nb].rearrange(
                            "p a (r c) -> p a r c", c=42)
                        dst = cur2[:, r0:r0 + nr + nr2, 0:42].rearrange(
                            "p (a r) c -> p a r c", a=2)
                        evac(dst, src_, b1, 2 * nb)

                def emit_l4_range(q0, nq):
                    """L4 DoubleRow matmuls for pairs q0..q0+nq-1 + 1
                    evacuation + de-pair DMAs + gpsimd square-norms."""
                    cur4 = l4in[(q0 // 10) % 2]
                    sl0 = q0 % 10
                    ps4 = pl34.tile([128, 512], f32, tag="ps34", name="ps4")
                    nb4 = 36 * nq
                    for g, (t0, t1) in enumerate(TAP_PAIRS):
                        nc.tensor.matmul(
                            ps4[:, 0:nb4], lhsT=w4[:, 2 * g:2 * g + 2, :],
                            rhs=dr_rhs(cur4, t0, t1, 6, 6 * nq, col0=12 * sl0),
                            start=(g == 0), stop=(g == 4), perf_mode=DR,
                        )
                    src4 = ps4[:, 0:nb4].rearrange(
                        "p (r q c) -> p q r c", r=6, c=6)
                    dst4 = eflat[:, q0:q0 + nq, :].rearrange(
                        "p q (r c) -> p q r c", c=6)
                    evac(dst4, src4, b4, nb4)
                    # de-pair this range into eall while the conv loop continues
                    nc.sync.dma_start(
                        out=eall[:, 2 * q0:2 * q0 + 2 * nq:2, :],
                        in_=eflat[0:64, q0:q0 + nq, :])
                    nc.sync.dma_start(
                        out=eall[:, 2 * q0 + 1:2 * q0 + 2 * nq:2, :],
                        in_=eflat[64:128, q0:q0 + nq, :])
                    esl = eall[:, 2 * q0:2 * q0 + 2 * nq, :]
                    sqt = sqp.tile([64, 20, 36], f32, tag="sqt", name="sqt")
                    nc.gpsimd.tensor_mul(sqt[:, :2 * nq, :], esl, esl)
                    load["dve"] += 1.0417 * 36 * 2 * nq + 125.0
                    nc.vector.reduce_sum(
                        out=sqr[:, 2 * q0:2 * q0 + 2 * nq],
                        in_=sqt[:, :2 * nq, :], axis=AX.X)

                def emit_gram1():
                    """Gram stage 1 over images 0-59 (octets 0-2 de-paired)
                    + support-prototype norm chain - all off the tail."""
                    psgt = pg.tile([25, 128], f32, tag="psg", name="psg")
                    gram_ps[0] = psgt
                    for s in range(36):
                        nc.tensor.matmul(
                            psgt[:, 0:60],
                            lhsT=eall[:, 0:S, s],
                            rhs=eall[:, 0:60, s],
                            start=(s == 0), stop=(s == 35),
                        )
                    nc.scalar.copy(out=gs[:, 0:60], in_=psgt[:, 0:60])
                    psa2 = psgt[0:5, 100:125]
                    nc.tensor.matmul(psa2, lhsT=oh5, rhs=gs[:, 0:S],
                                     start=True, stop=True)
                    a2s = sqp.tile([5, S], f32, tag="a2s", name="a2s")
                    nc.vector.tensor_mul(a2s, psa2, ohT5)
                    nc.vector.reduce_sum(out=np2, in_=a2s, axis=AX.X)
                    npv = sqp.tile([5, 1], f32, tag="npv", name="npv")
                    nc.scalar.sqrt(npv, np2)
                    npc_ = sqp.tile([5, 1], f32, tag="npc", name="npc")
                    nc.vector.tensor_scalar_max(npc_, npv, 1e-8)
                    nc.vector.reciprocal(invp, npc_)

                def emit_gram2():
                    """Gram stage 2 over images 60-99 (small tail stage)."""
                    psgt = gram_ps[0]
                    for s in range(36):
                        nc.tensor.matmul(
                            psgt[:, 60:100],
                            lhsT=eall[:, 0:S, s],
                            rhs=eall[:, 60:100, s],
                            start=(s == 0), stop=(s == 35),
                        )
                    nc.scalar.copy(out=gs[:, 60:100], in_=psgt[:, 60:100])

                def emit_l2(p):
                    """L2 DoubleRow matmuls for pair p into the trio PSUM tile;
                    per completed trio: merged evacuation + L3 + L4 cascade."""
                    nonlocal next_h
                    cur2 = l2in[p % 2]
                    g3, q3 = divmod(p, 3)
                    ps2 = pl2.tile([128, 512], f32, tag="ps2", name="ps2")
                    for g, (t0, t1) in enumerate(TAP_PAIRS):
                        nc.tensor.matmul(
                            ps2[:, :441], lhsT=w2[:, 2 * g:2 * g + 2, :],
                            rhs=dr_rhs(cur2, t0, t1, 21, 21),
                            start=(g == 0), stop=(g == 4), perf_mode=DR,
                        )
                    cur3 = l3in[g3 % 2]
                    src2 = ps2[:, :441].rearrange("p (r c) -> p r c", c=21)
                    dst2 = cur3[:, 1:22, 22 * q3 + 1:22 * q3 + 22]
                    evac(dst2, src2, b2, 441)
                    if not (q3 == 2 or p == NPAIR - 1):
                        return
                    nq = q3 + 1
                    # ---- L3 for this trio ----
                    ncol3 = 11 * nq
                    ps3 = pl34.tile([128, 512], f32, tag="ps34", name="ps3")
                    for g, (t0, t1) in enumerate(TAP_PAIRS):
                        nc.tensor.matmul(
                            ps3[:, 0:11 * ncol3],
                            lhsT=w3[:, 2 * g:2 * g + 2, :],
                            rhs=dr_rhs(cur3, t0, t1, 11, ncol3),
                            start=(g == 0), stop=(g == 4), perf_mode=DR,
                        )
                    src3 = ps3[:, 0:11 * ncol3].rearrange(
                        "p (r gc) -> p r gc", gc=ncol3)
                    # group trio pairs by their l4in buffer (octet parity)
                    runs = []
                    for q in range(nq):
                        pair = 3 * g3 + q
                        h = pair // 10
                        if runs and runs[-1][0] == h:
                            runs[-1][2] += 1
                        else:
                            runs.append([h, q, 1])
                    for h, q0, n in runs:
                        sl0 = (3 * g3 + q0) % 10
                        s_ = src3.rearrange(
                            "p r (q c) -> p q r c", c=11)[:, q0:q0 + n, :, :]
                        d_ = l4in[h % 2][:, 1:12, 12 * sl0:12 * (sl0 + n)]
                        d_ = d_.rearrange(
                            "p r (q c) -> p q r c", c=12)[:, :, :, 1:12]
                        evac(d_, s_, b3, n * 121)
                    # ---- L4 per completed range ----
                    pe = 3 * g3 + q3
                    while next_h < len(L4_RANGES) and L4_RANGES[next_h][2] <= pe:
                        q0r, nqr, _ = L4_RANGES[next_h]
                        emit_l4_range(q0r, nqr)
                        next_h += 1
                        if q0r + nqr == 30:
                            emit_gram1()
                        elif q0r + nqr == 50:
                            emit_gram2()

                # Software-pipelined emission: L1(p) is issued before L2(p-1)
                # so the tensor engine always has independent work while the
                # previous pair's PSUM is still being evacuated.
                CHUNKS = [1, 1, 2, 2] + [4] * 11   # pair counts per DMA; sum=50
                starts = []
                s0 = 0
                for n in CHUNKS:
                    starts.append(s0)
                    s0 += n
                chunk_of = {}
                for ci, (st, n) in enumerate(zip(starts, CHUNKS)):
                    for q in range(n):
                        chunk_of[st + q] = (ci, st, n)
                imtiles = {}

                def fetch_chunk(ci):
                    if ci >= len(CHUNKS) or ci in imtiles:
                        return
                    st, n = starts[ci], CHUNKS[ci]
                    imtile = imp.tile([28, 4, 3528], fp8, tag="im",
                                      name="imt")
                    imtiles[ci] = imtile
                    nc.sync.dma_start(
                        out=imtile[:, :n, :],
                        in_=d_im[st:st + n].transpose([1, 0, 2]),
                    )

                for p in range(NPAIR):
                    ci, st, n = chunk_of[p]
                    fetch_chunk(ci)
                    if p == st:
                        fetch_chunk(ci + 1)
                        if n <= 2:
                            fetch_chunk(ci + 2)
                    imtile = imtiles[ci]
                    emit_l1(p, imtile, p - st)
                    for tdst, tsrc, q in late_dmas.get(p, ()):
                        if q == "s":
                            nc.scalar.dma_start(out=tdst, in_=tsrc)
                        else:
                            nc.gpsimd.dma_start(out=tdst, in_=tsrc)
                    if p > 0:
                        emit_l2(p - 1)
                emit_l2(NPAIR - 1)

            # ---- head tail (gram + proto norms already done in-loop) ----
            with tc.tile_pool(name="hs", bufs=1) as hs, \
                 tc.tile_pool(name="ph", bufs=1, space="PSUM") as ph:
                psn = ph.tile([1, T], f32, tag="nt")
                nc.tensor.matmul(psn, lhsT=ones64, rhs=sqr[:, S:NIMG],
                                 start=True, stop=True)
                psdp = ph.tile([5, T], f32, tag="dp")
                nc.tensor.matmul(psdp, lhsT=oh5, rhs=gs[:, S:NIMG],
                                 start=True, stop=True)
                ntv = hs.tile([1, T], f32, tag="ntv")
                nc.scalar.sqrt(ntv, psn)
                ntc = hs.tile([1, T], f32, tag="ntc")
                nc.vector.tensor_scalar_max(ntc, ntv, 1e-8)
                invt = hs.tile([1, T], f32, tag="invt")
                nc.vector.reciprocal(invt, ntc)
                psr = ph.tile([5, T], f32, tag="rep")
                nc.tensor.matmul(psr, lhsT=ones15, rhs=invt, start=True, stop=True)
                invtr = hs.tile([5, T], f32, tag="invtr")
                nc.scalar.copy(out=invtr, in_=psr)
                pr1 = hs.tile([5, T], f32, tag="pr1")
                nc.vector.tensor_scalar(
                    out=pr1, in0=psdp, scalar1=invp, scalar2=None, op0=ALU.mult)
                pr2 = hs.tile([5, T], f32, tag="pr2")
                nc.vector.tensor_mul(pr2, pr1, invtr)
                nc.sync.dma_start(out=d_out, in_=pr2)

    nc.compile()
    return nc


def _host_prep(inputs):
    """Build per-core input maps (host-side layout transforms only)."""
    import ml_dtypes
    f8 = ml_dtypes.float8_e4m3
    f32 = np.float32
    xs = np.asarray(inputs["x_support_set"], f32)   # [8, 25, 3, 84, 84]
    xt = np.asarray(inputs["x_target_set"], f32)    # [8, 75, 3, 84, 84]
    y = np.asarray(inputs["y_support_set"])         # [8, 25] int32
    W1 = np.asarray(inputs["W1"], f32)

    # L1 weights: rows (dy, dx, ci) -> cols co; block diag for the image
    # pair; rows split into two k-tiles of 28 (rows 54, 55 zero).
    w1r = W1.transpose(2, 3, 1, 0).reshape(27, 64)
    wl1 = np.zeros((56, 128), f32)
    wl1[0:27, 0:64] = w1r
    wl1[27:54, 64:128] = w1r
    wl1 = np.ascontiguousarray(
        wl1.reshape(2, 28, 128).transpose(1, 0, 2)).astype(f8)

    def blockdiag(W):
        Wt = W.transpose(2, 3, 1, 0).reshape(9, 64, 64)  # [tap, ci, co]
        bd = np.zeros((10, 128, 128), f32)
        bd[0:9, 0:64, 0:64] = Wt
        bd[0:9, 64:128, 64:128] = Wt
        return np.ascontiguousarray(bd.transpose(1, 0, 2)).astype(f8)

    w2bd = blockdiag(np.asarray(inputs["W2"], f32))
    w3bd = blockdiag(np.asarray(inputs["W3"], f32))
    w4bd = blockdiag(np.asarray(inputs["W4"], f32))
    b1 = np.tile(np.asarray(inputs["b1"], f32), 2).reshape(128, 1)
    b2 = np.tile(np.asarray(inputs["b2"], f32), 2).reshape(128, 1)
    b3 = np.tile(np.asarray(inputs["b3"], f32), 2).reshape(128, 1)
    b4 = np.tile(np.asarray(inputs["b4"], f32), 2).reshape(128, 1)

    in_maps = []
    for c in range(B):
        x = np.concatenate([xs[c], xt[c]], 0)  # [100, 3, 84, 84]
        xp = np.zeros((NIMG, 3, 85, 85), f32)
        xp[:, :, :84, :84] = x
        win = np.lib.stride_tricks.sliding_window_view(xp, (3, 3), axis=(2, 3))
        w2v = win[:, :, ::2, ::2, :, :]                  # [100, 3, 42, 42, 3, 3]
        im = w2v.transpose(0, 4, 5, 1, 2, 3).reshape(NIMG, 27, 1764)
        im56 = np.zeros((NPAIR, 56, 1764), f32)
        im56[:, 0:27] = im[0::2]
        im56[:, 27:54] = im[1::2]
        im2row = np.ascontiguousarray(
            im56.reshape(NPAIR, 2, 28, 1764).transpose(0, 2, 1, 3)
        ).astype(f8).reshape(NPAIR, 28, 3528)

        onehot = (np.asarray(y[c]) % C)[:, None] == np.arange(C)[None, :]
        oh5 = (onehot.astype(f32) / C)
        in_maps.append({
            "im2row": im2row,
            "wl1": wl1, "w2bd": w2bd, "w3bd": w3bd, "w4bd": w4bd,
            "b1": b1, "b2": b2, "b3": b3, "b4": b4,
            "oh5": np.ascontiguousarray(oh5),
            "ohT5": np.ascontiguousarray(oh5.T),
        })
    return in_maps


def kernel(**inputs):
    from concourse import bass_utils

    if "nc" not in _CACHE:
        _CACHE["nc"] = _build_nc()
    nc = _CACHE["nc"]
    in_maps = _host_prep(inputs)
    res = bass_utils.run_bass_kernel_spmd(nc, in_maps, core_ids=list(range(B)))
    preds = np.stack([r["preds"] for r in res.results], 0)  # [8, 5, 75]
    return np.ascontiguousarray(preds.transpose(0, 2, 1)).astype(np.float32)
